# revision 71
# baseline (speedup 1.0000x reference)
"""DualMem retrieval kernel for Trainium2 (8 NeuronCores, Bass/Tile).

Math (per reference):
    sim[b,c,m]  = <img[b], mem[c,m]>
    w           = exp(-beta * (1 - sim))
    adapt[b,c]  = sum_m mem[c,m] * w[b,c,m]
    logits[b,c] = 100 * <img[b], adapt[b,c] / ||adapt[b,c]||>

Key algebraic reduction (avoids materializing adapt [B,C,D]):
    numer[b,c]  = <img[b], adapt[b,c]> = sum_m w[b,c,m] * sim[b,c,m]
    denom[b,c]  = ||adapt[b,c]||^2     = w^T G_c w,  G_c = mem_c @ mem_c^T  (11x11 Gram)
    logits      = 100 * numer / sqrt(denom)

Sharding: classes C=1000 split 125 per core across 8 cores (mem bank fully
sharded; only img replicated).

Per-core layout: groups of 11 classes x 11 memories = 121 partitions (pad to
128); 12 groups cover 132 >= 125 class slots.  The bf16 memory bank arrives
class-major and is xbar-DMA-transposed on-chip to [d, cm]; img, the Gram
mask, and the 0/1 class-sum matrix ride the same transpose stream (all
bf16-exact), so there are no plain input DMAs at all.  Groups are processed
in blocks of [4,4,3,1] sharing PSUM banks (per-element has_written makes
disjoint column ranges in one bank legal accumulation groups; the bank-level
software check is skipped):
    su bank [128, gn*128]: per group k, cols 128k+0:64  = sim (acc over d)
                                        cols 128k+64:128 = u = G_masked^T w
    G bank  [128, gn*128]: per group k, cols 128k:128k+128 = Gram (acc over d)
Downstream per block: one batched exp, one masked-Gram copy (the mandatory
PSUM->SBUF move), gn u-matmuls, one broadcast mul building [w*sim | w*u],
one 0/1 "E" matmul summing over m per class -> [numer | denom] in PSUM.
Finals read PSUM directly and use 100/sqrt(d) = exp(-0.5*ln(d) + ln(100));
Ln and Exp share one ACT function table, so the table is loaded exactly
once.  The small trailing blocks keep the end-of-kernel dependency chain
short, per-block finals overlap earlier compute, each block's sim/G matmuls
are emitted ahead of the previous block's downstream so the PE queue favors
them, and dependency-free junk matmuls warm the PE (HAM clock gate) during
the transpose startup window.
"""

import sys

sys.path.insert(0, "/opt/trn_rl_repo")

import ml_dtypes
import numpy as np

B, C, M, D = 64, 1000, 11, 1024
BETA = 5.5
N_CORES = 8
C_PER = C // N_CORES          # 125 classes per core
CPG = 11                      # classes per group
NG = 12                       # groups per core (132 class slots >= 125)
PG = CPG * M                  # 121 used partitions per group
DCH = D // 128                # 8 d-chunks
ROWS = NG * 128               # 1536 class-major rows per core

_cache = {}


def _build():
    import concourse.mybir as mybir
    import concourse.tile as tile
    from concourse import bacc

    # Pin every activation to the one ACT table that holds BOTH Exp and Ln
    # (indices must be preserved — empty the other sets instead of dropping
    # them) so the function table is loaded once and never swapped.
    if not getattr(bacc, "_act_tables_pinned", False):
        real = bacc.get_activation_tables

        def pinned(arch):
            return {k: (v if k == "natural_log_exp_and_others" else set())
                    for k, v in real(arch).items()}
        bacc.get_activation_tables = pinned
        bacc._act_tables_pinned = True

    f32 = mybir.dt.float32
    bf16 = mybir.dt.bfloat16

    nc = bacc.Bacc("TRN2", target_bir_lowering=False, debug=False,
                   num_devices=N_CORES)

    # membf rows: [64 img | 128 mask/em | 1536 class-major memory rows].
    # The xbar transpose of the leading rows lands imgT, the block-diagonal
    # Gram mask, and the 0/1 class-sum matrix (all bf16-exact) in exactly
    # the layouts the compute wants — no separate const loads at all.
    EXT = 192
    membf = nc.dram_tensor("membf", [EXT + 11 * 128 + 48, D], bf16,
                           kind="ExternalInput")
    out = nc.dram_tensor("out", [16, NG * 64], f32, kind="ExternalOutput")

    with tile.TileContext(nc) as tc:
        with (
            tc.tile_pool(name="const", bufs=1) as const,
            tc.tile_pool(name="sb", bufs=3) as sb,
            tc.tile_pool(name="ps_su", bufs=2, space="PSUM") as ps_su,
            tc.tile_pool(name="ps_g", bufs=2, space="PSUM") as ps_g,
            tc.tile_pool(name="ps_nd", bufs=1, space="PSUM") as ps_nd,
        ):
            # memT[d % 128, d_chunk, cm]; transpose batches sized so compute
            # can start right after img+g0 land:
            #   b0: img(64)+g0(128)  b1: mask/em(128)  b2: g1  b3: g2,g3
            #   b4..b7: g4..g11 two groups each
            bat_rows = [192, 128, 128, 256, 256, 128, 128, 128, 128, 128, 48]
            mt = [const.tile([128, 5 if q == 1 else DCH, r], bf16,
                             name=f"mt{q}", tag=f"mt{q}")
                  for q, r in enumerate(bat_rows)]
            # batch 0 is transposed as two column-halves into separate
            # tiles so g0's first d-chunks are compute-ready after half
            # the data
            mt0 = [const.tile([128, 4, 192], bf16, name=f"mt0{h}",
                              tag=f"mt0{h}") for h in range(2)]
            # group g -> (batch tile, col offset); g0 handled via mt0
            gloc = {0: (None, 64), 1: (mt[2], 0), 2: (mt[3], 0),
                    3: (mt[3], 128)}
            for g in range(4, 6):
                gloc[g] = (mt[4 + (g - 4) // 2], 128 * ((g - 4) % 2))
            gloc[5] = (mt[4], 128)
            for g in range(6, NG):
                gloc[g] = (mt[5 + (g - 6)], 0)

            def it_chunk(i):
                return mt0[i // 4][:, i % 4, 0:64]

            def blk_chunk(g, i, off, gw):
                if g == 0:
                    return mt0[i // 4][:, i % 4, 64:64 + gw]
                tile_, o = gloc[g]
                return tile_[:, i, o + (off - o):o + (off - o) + gw]
            mask_bf = mt[1][:, 0:4, :]               # [128, 4, 128] bf16
            em_bf = mt[1][:, 4, 0:16]                # [128, 16] bf16
            lg = const.tile([16, NG * 64], f32)
            bias_exp = const.tile([128, 1], f32)
            bias_eps = const.tile([16, 1], f32)
            bias_ln100 = const.tile([16, 1], f32)
            junk_w = const.tile([128, 16], bf16)
            junk_x = const.tile([128, 512], bf16)
            nc.vector.memset(junk_w[:], 0)
            nc.vector.memset(junk_x[:], 0)
            nc.vector.memset(bias_exp[:], -BETA)
            nc.vector.memset(bias_eps[:], 1e-30)
            nc.vector.memset(bias_ln100[:], float(np.log(100.0)))

            # xbar transposes in issue order; everything (img, mask/em, mem
            # bank) rides the transpose stream — no plain input DMAs at all.
            r0 = 0
            for q, r in enumerate(bat_rows):
                if q == 0:
                    for h in range(2):
                        nc.sync.dma_start(
                            mt0[h][:],
                            membf.ap()[0:192, h * 512:(h + 1) * 512],
                            transpose=True,
                        )
                else:
                    ncol = 5 * 128 if q == 1 else D
                    nc.sync.dma_start(
                        mt[q][:],
                        membf.ap()[r0:r0 + r, 0:ncol],
                        transpose=True,
                    )
                r0 += r

            # [numer | denom]: blocks 0-1 share a 2-bank PSUM tile so their
            # finals run while block 2 computes; block 2 gets its own bank
            nd_a = ps_nd.tile([16, 2 * GPB * 128], f32, name="nd_a")
            nd_b = ps_nd.tile([16, GPB * 128], f32, name="nd_b")

            exps = []
            for nb in range(NB):
                su = ps_su.tile([128, GPB * 128], f32)
                gp = ps_g.tile([128, GPB * 128], f32)
                for k in range(GPB):
                    tile_, off = gloc[nb * GPB + k]
                    for i in range(DCH):
                        blk = tile_[:, i, off:off + 128]
                        nc.tensor.matmul(su[:, k * 128:k * 128 + 64],
                                         blk, it[:, i, :],
                                         start=(i == 0), stop=(i == DCH - 1),
                                         skip_group_check=True)
                        nc.tensor.matmul(gp[:, k * 128:(k + 1) * 128],
                                         blk, blk,
                                         start=(i == 0), stop=(i == DCH - 1),
                                         skip_group_check=True)

                # w = exp(beta*sim - beta) for all 4 groups at once
                su4 = su[:].rearrange("p (k t b) -> p k t b", k=GPB, t=2)
                w4 = sb.tile([128, GPB * 64], bf16, tag="w4")
                exps.append(nc.scalar.activation(
                    w4[:], su4[:, :, 0, :],
                    mybir.ActivationFunctionType.Exp,
                    bias=bias_exp[:], scale=BETA))

                # masked Gram -> SBUF (kills cross-class + pad entries)
                gm4 = sb.tile([128, GPB * 128], bf16, tag="gm4")
                gp4 = gp[:].rearrange("p (k j) -> p k j", k=GPB)
                nc.vector.tensor_mul(gm4[:], gp4, mask_bf)

                # u_k = G_k^T @ w_k, placed next to sim_k in the same bank
                for k in range(GPB):
                    nc.tensor.matmul(su[:, k * 128 + 64:(k + 1) * 128],
                                     gm4[:, k * 128:(k + 1) * 128],
                                     w4[:, k * 64:(k + 1) * 64],
                                     start=True, stop=True,
                                     skip_group_check=True)

                # wsq = [w*sim | w*u], one fused mul with w broadcast over t
                wsq = sb.tile([128, GPB * 128], bf16, tag="wsq")
                wq4 = wsq[:].rearrange("p (k t b) -> p k t b", k=GPB, t=2)
                w4b = w4[:].rearrange("p (k u b) -> p k u b", k=GPB, u=1) \
                    .to_broadcast((128, GPB, 2, 64))
                nc.vector.tensor_mul(wq4, su4, w4b)

                # nd[c, :] = [numer | denom] per class for the whole block
                dst = (nd_a[:, nb * GPB * 128:(nb + 1) * GPB * 128]
                       if nb < 2 else nd_b[:])
                nc.tensor.matmul(dst, em_bf, wsq[:], start=True, stop=True,
                                 skip_group_check=True)

            # logits = numer * 100/sqrt(denom), straight out of PSUM;
            # blocks 0-1 finalize while block 2 still computes
            # 100/sqrt(denom) = exp(-0.5*ln(denom) + ln(100)) -- Ln and Exp
            # live in the same ACT function table, so no table swap ever.
            # Compute-only; all output DMAs are emitted at the very end so
            # they can never be scheduled among the xbar transposes.
            def emit_final(half):
                nd_t, n, go = [(nd_a, 8, 0), (nd_b, 3, 8), (nd_c, 1, 11)][half]
                nd3 = nd_t[:].rearrange("p (g t b) -> p g t b", g=n, t=2)
                s_h = sb.tile([16, n * 64], f32, tag=f"s{half}",
                              name=f"s_{half}")
                nc.scalar.activation(s_h[:], nd3[:, :, 1, :],
                                     mybir.ActivationFunctionType.Ln,
                                     bias=bias_eps[:], scale=1.0)
                r_h = sb.tile([16, n * 64], f32, tag=f"r{half}",
                              name=f"r_{half}")
                nc.scalar.activation(r_h[:], s_h[:],
                                     mybir.ActivationFunctionType.Exp,
                                     bias=bias_ln100[:], scale=-0.5)
                o0 = go * 64
                nc.vector.tensor_mul(lg[:, o0:o0 + n * 64], nd3[:, :, 0, :],
                                     r_h[:])
            emit_final(1)
            emit_final(2)
            nc.sync.dma_start(out.ap()[:, 0:512], lg[:, 0:512])
            nc.sync.dma_start(out.ap()[:, 512:768], lg[:, 512:768])

    nc.compile()
    return nc


def _get_nc():
    if "nc" not in _cache:
        _cache["nc"] = _build()
    return _cache["nc"]


def _prep_inputs(img_features, memorized_image_feat):
    """Host-side formatting: bf16 cast, class padding, group layout."""
    bf = ml_dtypes.bfloat16
    img_b = np.ascontiguousarray(img_features.astype(bf))          # [64, 1024]
    mem_b = memorized_image_feat.astype(bf)                        # [1000,11,1024]

    m1 = np.zeros((128, 128), np.float32)
    for c in range(CPG):
        m1[c * M:(c + 1) * M, c * M:(c + 1) * M] = 1.0
    em = np.zeros((128, 16), np.float32)
    for c in range(CPG):
        em[c * M:(c + 1) * M, c] = 1.0

    # mask/em rows for the transpose stream: transposing maskem[j, 128i+p]
    # yields m1 at d-chunks 0-3 and em^T at chunk 4
    maskem = np.zeros((128, D), bf)
    for i in range(4):
        maskem[:, i * 128:(i + 1) * 128] = m1.T
    maskem[:16, 512:640] = em.T

    in_maps = []
    for k in range(N_CORES):
        sl = mem_b[k * C_PER:(k + 1) * C_PER]                      # [125,11,1024]
        pad = np.zeros((NG * CPG, M, D), bf)
        pad[:C_PER] = sl
        grp = pad.reshape(NG, PG, D)
        full = np.zeros((NG, 128, D), bf)
        full[:, :PG] = grp
        rows = full.reshape(ROWS, D)
        nrows = 192 + 11 * 128 + 48
        membf = np.empty((nrows, D), bf)
        membf[:64] = img_b              # batch 0: img + g0
        membf[64:192] = rows[:128]
        membf[192:320] = maskem         # batch 1: mask/em
        membf[320:320 + 10 * 128] = rows[128:11 * 128]  # g1..g10
        membf[320 + 10 * 128:] = rows[11 * 128:11 * 128 + 48]  # g11 short
        in_maps.append({"membf": membf})
    return in_maps


def _gather(results):
    logits = np.empty((B, C), np.float32)
    for k in range(N_CORES):
        o = results[k]["out"].reshape(16, NG, 64)[:CPG]            # [11, 12, 64]
        o = o.transpose(1, 0, 2).reshape(NG * CPG, 64)[:C_PER]     # [125, 64]
        logits[:, k * C_PER:(k + 1) * C_PER] = o.T
    return logits


def kernel(img_features, memorized_image_feat):
    from concourse.bass_utils import run_bass_kernel_spmd

    nc = _get_nc()
    in_maps = _prep_inputs(img_features, memorized_image_feat)
    res = run_bass_kernel_spmd(nc, in_maps, core_ids=list(range(N_CORES)))
    return _gather(res.results)
